# revision 11
# baseline (speedup 1.0000x reference)
"""Correlation layer for 8 Trainium2 NeuronCores — Bass/Tile kernel.

Data-parallel over batch: 16 samples -> 2 per core. Per (sample, h-row) the
kernel computes Gram tiles G[w, dy, w'] = sum_c f1[c,w] * f2pad[c, h+dy, w']
on the TensorEngine (C=256 contracted in two 128-chunks, bf16), evacuates
them into an SBUF buffer laid out [w, dy, w', h] (DVE, scaled by 1/C), dumps
that to a DRAM scratch, and finally extracts the 9 diagonals per dy with a
single strided DRAM->DRAM DMA per h-block (the diagonal G[w, w+dx] is affine
in DRAM: w-stride = D*WP*HB + HB). Output is produced as [b, ch, hb, w, hl]
and untransposed on the host.
"""
import numpy as np

B, C, H, W = 16, 256, 64, 128
PAD, D = 4, 9
BB = 2               # samples per core
N_CORES = 8
HP, WP = H + 2 * PAD, W + 2 * PAD   # 72, 136
HB = 16              # h-block size in the Gall buffer
NHB = H // HB        # 4
CH = D * D           # 81

_cache = {}


def _build():
    import concourse.bass as bass
    import concourse.bacc as bacc
    import concourse.mybir as mybir
    from concourse.tile import TileContext

    f32, bf16 = mybir.dt.float32, mybir.dt.bfloat16
    nc = bacc.Bacc()
    f1 = nc.declare_dram_parameter("f1", [BB, C, H, W], f32, isOutput=False)
    f2 = nc.declare_dram_parameter("f2", [BB, C, H, W], f32, isOutput=False)
    outT = nc.declare_dram_parameter(
        "outT", [BB, CH, NHB, W, HB], f32, isOutput=True
    )

    with TileContext(nc) as tc:
        with (
            tc.tile_pool(name="feat", bufs=1) as feat,
            tc.tile_pool(name="gall", bufs=2) as gall_pool,
            tc.tile_pool(name="scr", bufs=2, space="DRAM") as scr_pool,
            tc.tile_pool(name="ps", bufs=2, space="PSUM") as ps,
        ):
            for b in range(BB):
                F1 = feat.tile([128, 2, H, W], bf16, tag="F1")
                F2 = feat.tile([128, 2, HP, WP], bf16, tag="F2")
                f1r = f1[b].rearrange("(k p) h w -> p k h w", p=128)
                f2r = f2[b].rearrange("(k p) h w -> p k h w", p=128)
                nc.any.memzero(F2[:])
                for k in range(2):
                    nc.gpsimd.dma_start(out=F1[:, k], in_=f1r[:, k])
                    nc.gpsimd.dma_start(
                        out=F2[:, k, PAD:PAD + H, PAD:PAD + W], in_=f2r[:, k]
                    )
                for hb in range(NHB):
                    G = gall_pool.tile([128, D, WP, HB], bf16, tag="G")
                    for hl in range(HB):
                        h = hb * HB + hl
                        for t in range(3):
                            pt = ps.tile([128, 3, WP], f32, tag=f"pt{t}")
                            for k in range(2):
                                nc.tensor.matmul(
                                    pt[:],
                                    F1[:, k, h, :],
                                    F2[:, k, h + 3 * t: h + 3 * t + 3, :],
                                    start=(k == 0),
                                    stop=(k == 1),
                                )
                            nc.vector.tensor_copy(
                                G[:, 3 * t:3 * t + 3, :, hl], pt[:]
                            )
                    scr = scr_pool.tile([128, D, WP, HB], bf16, tag="scr")
                    nc.sync.dma_start(out=scr[:], in_=G[:])
                    # Diagonal extraction: dst[dy, dx, w, hl] = scr[w, dy, w+dx, hl]
                    dst5 = outT[b].rearrange(
                        "(dy dx) nhb w hl -> dy dx nhb w hl", dy=D
                    )[:, :, hb]
                    for dx in range(D):
                        diag = bass.AP(
                            scr.tensor,
                            scr.offset + dx * HB,
                            [
                                [WP * HB, D],            # dy
                                [D * WP * HB + HB, W],   # w (+HB = diagonal skew)
                                [1, HB],                 # hl
                            ],
                        )
                        nc.gpsimd.dma_start(out=dst5[:, dx], in_=diag)
    if not nc.is_finalized():
        nc.finalize()
    return nc


def _get_built():
    if "nc" not in _cache:
        _cache["nc"] = _build()
    return _cache["nc"]


def kernel(features1, features2):
    from concourse.bass_utils import run_bass_kernel_spmd

    features1 = np.ascontiguousarray(np.asarray(features1, dtype=np.float32))
    features2 = np.ascontiguousarray(np.asarray(features2, dtype=np.float32))
    nc = _get_built()
    in_maps = [
        {
            "f1": features1[i * BB:(i + 1) * BB],
            "f2": features2[i * BB:(i + 1) * BB],
        }
        for i in range(N_CORES)
    ]
    res = run_bass_kernel_spmd(nc, in_maps, list(range(N_CORES)))
    shards = []
    for i in range(N_CORES):
        oT = res.results[i]["outT"]  # [BB, CH, NHB, W, HB]
        shards.append(
            np.transpose(oT, (0, 1, 2, 4, 3)).reshape(BB, CH, H, W)
        )
    out = np.concatenate(shards, axis=0)
    out *= 1.0 / C  # mean over channels
    return out.astype(np.float32)


def time_kernel_ns(features1, features2, n_small=4, n_big=24):
    """Per-execution device time via async-dispatch slope."""
    import time
    import jax
    import numpy as np
    from jax.sharding import Mesh, PartitionSpec, NamedSharding
    from jax.experimental.shard_map import shard_map
    import concourse.mybir as mybir
    from concourse import bass2jax

    nc = _get_built()
    bass2jax.install_neuronx_cc_hook()

    part_name = nc.partition_id_tensor.name if nc.partition_id_tensor else None
    in_names, out_names, out_avals, zero_outs = [], [], [], []
    for alloc in nc.m.functions[0].allocations:
        if not isinstance(alloc, mybir.MemoryLocationSet):
            continue
        name = alloc.memorylocations[0].name
        if alloc.kind == "ExternalInput":
            if name != part_name:
                in_names.append(name)
        elif alloc.kind == "ExternalOutput":
            out_names.append(name)
            shape = tuple(alloc.tensor_shape)
            dtype = mybir.dt.np(alloc.dtype)
            out_avals.append(jax.core.ShapedArray(shape, dtype))
            zero_outs.append(np.zeros(shape, dtype))
    n_params = len(in_names)
    all_names = in_names + out_names
    if part_name is not None:
        all_names.append(part_name)

    def _body(*args):
        operands = list(args)
        if part_name is not None:
            operands.append(bass2jax.partition_id_tensor())
        outs = bass2jax._bass_exec_p.bind(
            *operands,
            out_avals=tuple(out_avals),
            in_names=tuple(all_names),
            out_names=tuple(out_names),
            lowering_input_output_aliases=(),
            sim_require_finite=True,
            sim_require_nnan=True,
            nc=nc,
        )
        return tuple(outs)

    devices = jax.devices()[:N_CORES]
    mesh = Mesh(np.asarray(devices), ("core",))
    nin = n_params + len(out_names)
    fn = jax.jit(
        shard_map(
            _body, mesh=mesh,
            in_specs=(PartitionSpec("core"),) * nin,
            out_specs=(PartitionSpec("core"),) * len(out_names),
            check_rep=False,
        ),
        keep_unused=True,
    )
    sh = NamedSharding(mesh, PartitionSpec("core"))
    f1 = np.ascontiguousarray(features1, dtype=np.float32)
    f2 = np.ascontiguousarray(features2, dtype=np.float32)
    args = [jax.device_put(f1, sh), jax.device_put(f2, sh)]
    args += [jax.device_put(np.zeros((N_CORES * z.shape[0],) + z.shape[1:],
                                     z.dtype), sh) for z in zero_outs]

    # warm (compiles)
    for _ in range(2):
        r = fn(*args)
        jax.block_until_ready(r)

    def run_n(n):
        t0 = time.perf_counter()
        r = None
        for _ in range(n):
            r = fn(*args)
        jax.block_until_ready(r)
        return time.perf_counter() - t0

    run_n(2)
    t_small = min(run_n(n_small) for _ in range(3))
    t_big = min(run_n(n_big) for _ in range(3))
    return (t_big - t_small) / (n_big - n_small) * 1e9


if __name__ == "__main__":
    rng = np.random.default_rng(0)
    a = rng.standard_normal((B, C, H, W), dtype=np.float32)
    bb = rng.standard_normal((B, C, H, W), dtype=np.float32)
    y = kernel(features1=a, features2=bb)
    print("out:", y.shape, y.dtype, float(np.abs(y).max()))


# revision 15
# speedup vs baseline: 1.0842x; 1.0842x over previous
"""Correlation layer for 8 Trainium2 NeuronCores — Bass/Tile kernel.

Data-parallel over batch: 16 samples -> 2 per core. Per (sample, h-row) the
kernel computes windowed Gram tiles on the TensorEngine with 4 column-tiled
M=32 matmuls (tile_position col groups, C=256 contracted in two 128-chunks,
bf16): psum[32s+i, dy, j] = sum_c f1[c, 32s+i] * f2pad[c, h+dy, 32s+j],
j in [0,40). The DVE evacuates each h-row (one [128, 9*40] copy) into an
SBUF buffer laid out [w, dy, j, h], which is dumped to DRAM scratch once per
sample. The needed correlation values sit on in-strip diagonals
(j = i + dx), which are affine in DRAM, so one strided DRAM->DRAM DMA per
(sample, dy) extracts them (innermost run = (dx,h) contiguous 576 elems)
into outT[b, ch, w, h]; the host untransposes (w,h) and applies the 1/C
mean scale.
"""
import numpy as np

B, C, H, W = 16, 256, 64, 128
PAD, D = 4, 9
BB = 2               # samples per core
N_CORES = 8
HP, WP = H + 2 * PAD, W + 2 * PAD   # 72, 136
SW = 32              # w-strip width (col-tile M)
NS = W // SW         # 4 strips
WIN = SW + 2 * PAD   # 40: f2 window per strip
CH = D * D           # 81

_cache = {}


def _build():
    import concourse.bass as bass
    import concourse.bacc as bacc
    import concourse.mybir as mybir
    from concourse.tile import TileContext

    f32, bf16 = mybir.dt.float32, mybir.dt.bfloat16
    nc = bacc.Bacc()
    f1 = nc.declare_dram_parameter("f1", [BB, C, H, W], f32, isOutput=False)
    f2 = nc.declare_dram_parameter("f2", [BB, C, H, W], f32, isOutput=False)
    # Output in diagonal-extraction order: [b, dy, s, i, dx, h]; host permutes.
    outT = nc.declare_dram_parameter(
        "outT", [BB, D, NS, SW, D, H], f32, isOutput=True
    )

    with TileContext(nc) as tc:
        with (
            tc.tile_pool(name="feat", bufs=1) as feat,
            tc.tile_pool(name="gall", bufs=2) as gall_pool,
            tc.tile_pool(name="scr", bufs=2, space="DRAM") as scr_pool,
            tc.tile_pool(name="ps", bufs=4, space="PSUM") as ps,
        ):
            for b in range(BB):
                F1 = feat.tile([128, 2, H, W], bf16, tag="F1")
                F2 = feat.tile([128, 2, HP, WP], bf16, tag="F2")
                f1r = f1[b].rearrange("(k p) h w -> p k h w", p=128)
                f2r = f2[b].rearrange("(k p) h w -> p k h w", p=128)
                nc.any.memzero(F2[:])
                for k in range(2):
                    nc.gpsimd.dma_start(out=F1[:, k], in_=f1r[:, k])
                    nc.gpsimd.dma_start(
                        out=F2[:, k, PAD:PAD + H, PAD:PAD + W], in_=f2r[:, k]
                    )
                # Gall[w, dy, j, h] bf16: 9*40*64*2 = 45KB/partition
                G = gall_pool.tile([128, D, WIN, H], bf16, tag="G")
                for h in range(H):
                    pt = ps.tile([128, D, WIN], f32, tag="pt")
                    for k in range(2):
                        for s in range(NS):
                            nc.tensor.matmul(
                                pt[s * SW:(s + 1) * SW],
                                F1[:, k, h, s * SW:(s + 1) * SW],
                                F2[:, k, h:h + D, s * SW:s * SW + WIN],
                                start=(k == 0),
                                stop=(k == 1),
                                tile_position=(0, s * SW),
                            )
                    nc.vector.tensor_copy(G[:, :, :, h], pt[:])
                scr = scr_pool.tile([128, D, WIN, H], bf16, tag="scr")
                nc.sync.dma_start(out=scr[:], in_=G[:])
                # Diagonal extraction, one DMA per dy:
                #   dst[s, i, dx, h] = scr[w=32s+i, dy, j=i+dx, h]
                # dst (= outT[b, dy]) is fully contiguous in that order.
                for dy in range(D):
                    src = bass.AP(
                        scr.tensor,
                        scr.offset + dy * WIN * H,
                        [
                            [SW * D * WIN * H, NS],      # s
                            [D * WIN * H + H, SW],       # i (+H = diagonal skew)
                            [1, D * H],                  # (dx, h) contiguous
                        ],
                    )
                    nc.gpsimd.dma_start(out=outT[b, dy], in_=src)
    if not nc.is_finalized():
        nc.finalize()
    return nc


def _get_built():
    if "nc" not in _cache:
        _cache["nc"] = _build()
    return _cache["nc"]


def kernel(features1, features2):
    from concourse.bass_utils import run_bass_kernel_spmd

    features1 = np.ascontiguousarray(np.asarray(features1, dtype=np.float32))
    features2 = np.ascontiguousarray(np.asarray(features2, dtype=np.float32))
    nc = _get_built()
    in_maps = [
        {
            "f1": features1[i * BB:(i + 1) * BB],
            "f2": features2[i * BB:(i + 1) * BB],
        }
        for i in range(N_CORES)
    ]
    res = run_bass_kernel_spmd(nc, in_maps, list(range(N_CORES)))
    shards = []
    for i in range(N_CORES):
        oT = res.results[i]["outT"]  # [BB, dy, s, i, dx, h]
        shards.append(
            np.transpose(oT, (0, 1, 4, 5, 2, 3)).reshape(BB, CH, H, W)
        )
    out = np.concatenate(shards, axis=0)
    out *= 1.0 / C  # mean over channels
    return np.ascontiguousarray(out).astype(np.float32)


def _pjrt_fn(nc):
    import jax
    import numpy as np
    from jax.sharding import Mesh, PartitionSpec
    from jax.experimental.shard_map import shard_map
    import concourse.mybir as mybir
    from concourse import bass2jax

    bass2jax.install_neuronx_cc_hook()
    part_name = nc.partition_id_tensor.name if nc.partition_id_tensor else None
    in_names, out_names, out_avals, zero_outs = [], [], [], []
    for alloc in nc.m.functions[0].allocations:
        if not isinstance(alloc, mybir.MemoryLocationSet):
            continue
        name = alloc.memorylocations[0].name
        if alloc.kind == "ExternalInput":
            if name != part_name:
                in_names.append(name)
        elif alloc.kind == "ExternalOutput":
            out_names.append(name)
            shape = tuple(alloc.tensor_shape)
            dtype = mybir.dt.np(alloc.dtype)
            out_avals.append(jax.core.ShapedArray(shape, dtype))
            zero_outs.append(np.zeros(shape, dtype))
    all_names = in_names + out_names + ([part_name] if part_name else [])

    def _body(*args):
        operands = list(args)
        if part_name is not None:
            operands.append(bass2jax.partition_id_tensor())
        outs = bass2jax._bass_exec_p.bind(
            *operands,
            out_avals=tuple(out_avals),
            in_names=tuple(all_names),
            out_names=tuple(out_names),
            lowering_input_output_aliases=(),
            sim_require_finite=True,
            sim_require_nnan=True,
            nc=nc,
        )
        return tuple(outs)

    devices = jax.devices()[:N_CORES]
    mesh = Mesh(np.asarray(devices), ("core",))
    nin = len(in_names) + len(out_names)
    fn = jax.jit(
        shard_map(
            _body, mesh=mesh,
            in_specs=(PartitionSpec("core"),) * nin,
            out_specs=(PartitionSpec("core"),) * len(out_names),
            check_rep=False,
        ),
        keep_unused=True,
    )
    return fn, mesh, zero_outs


def _slope_ns(fn, args, n_small=4, n_big=24):
    import time
    import jax

    for _ in range(2):
        jax.block_until_ready(fn(*args))

    def run_n(n):
        t0 = time.perf_counter()
        r = None
        for _ in range(n):
            r = fn(*args)
        jax.block_until_ready(r)
        return time.perf_counter() - t0

    run_n(2)
    t_small = min(run_n(n_small) for _ in range(3))
    t_big = min(run_n(n_big) for _ in range(3))
    return (t_big - t_small) / (n_big - n_small) * 1e9


def time_kernel_ns(features1, features2):
    """Per-execution device time via async-dispatch slope."""
    import jax
    from jax.sharding import PartitionSpec, NamedSharding

    nc = _get_built()
    fn, mesh, zero_outs = _pjrt_fn(nc)
    sh = NamedSharding(mesh, PartitionSpec("core"))
    f1 = np.ascontiguousarray(features1, dtype=np.float32)
    f2 = np.ascontiguousarray(features2, dtype=np.float32)
    args = [jax.device_put(f1, sh), jax.device_put(f2, sh)]
    args += [
        jax.device_put(
            np.zeros((N_CORES * z.shape[0],) + z.shape[1:], z.dtype), sh
        )
        for z in zero_outs
    ]
    return _slope_ns(fn, args)


if __name__ == "__main__":
    rng = np.random.default_rng(0)
    a = rng.standard_normal((B, C, H, W), dtype=np.float32)
    bb = rng.standard_normal((B, C, H, W), dtype=np.float32)
    y = kernel(features1=a, features2=bb)
    print("out:", y.shape, y.dtype, float(np.abs(y).max()))
